# revision 23
# baseline (speedup 1.0000x reference)
"""Trainium2 Bass kernel for LlamaAttention (B=1, S=2048, H=4096, 32 heads).

Tensor-parallel over heads: 8 cores x 4 heads. v4 design:
  - All matmuls bf16 (1 cycle/row, ldweights ~99ns fully hidden),
    accumulation in fp32 PSUM.
  - Wq/Wk/Wv/Wo slices resident in SBUF, loaded once during chunk 0.
  - Software pipeline across 512-wide seq chunks:
      chunk c projection passes (K -> rope-K, Q -> rope-Q, V) are emitted
      interleaved with attention(c-1) blocks and o_proj(c-2) tiles, so the
      tensor engine always has dense matmul work while exp (ACT), softmax
      sums (DVE) and rope (ACT+DMA+DVE) latencies play out on other engines.
      Tail: attention(3) x o_proj(2), then o_proj(3) on the freed acc ring.
  - RoPE rotate-half: ACT evict to SBUF + DMA partition shift + DVE combine.
  - Softmax: es accumulated on DVE, partition-reduced by a ones-matmul,
    1/sum broadcast by a rank-1 matmul (both inside the sc PSUM ring).
  - Causal staircase: diagonal blocks only mask/exp the valid columns.
  - po partials written bf16; host sums the 8 partials in fp32.
PSUM banks: acc=4 (kps/qps/vps + tail o_proj), sc=2, ov=1, pps=1.
"""

import os
import sys

if "/opt/trn_rl_repo" not in sys.path:
    sys.path.insert(0, "/opt/trn_rl_repo")

import numpy as np
import ml_dtypes

from concourse import bacc, mybir, tile
from concourse import bass
from concourse.bass_utils import run_bass_kernel_spmd

F32 = mybir.dt.float32
F32R = mybir.dt.float32r
BF16 = mybir.dt.bfloat16
EXPF = mybir.ActivationFunctionType.Exp

N_CORES = 8
S = 2048
H = 4096
N_HEADS = 32
D = 128
HPC = N_HEADS // N_CORES     # 4 heads per core
HC = HPC * D                 # 512
CH = 512
NCH = S // CH                # 4
KT_TILES = H // 128          # 32
SJT = S // 128               # 16
ROPE_BASE = 10000.0
NEG = -1.0e9

last_exec_time_ns = None

BFDT = ml_dtypes.bfloat16


def _r(x):
    return np.ascontiguousarray(x, dtype=np.float32)


def _b(x):
    return np.ascontiguousarray(np.asarray(x, dtype=np.float32), dtype=BFDT)


def _fair_merge(a, b):
    """Round-robin merge two emitter lists proportionally."""
    out = []
    na, nb = len(a), len(b)
    ia = ib = 0
    for _ in range(na + nb):
        if ib * na <= ia * nb and ib < nb:
            out.append(b[ib]); ib += 1
        elif ia < na:
            out.append(a[ia]); ia += 1
        else:
            out.append(b[ib]); ib += 1
    return out


def _build(causal: bool):
    nc = bacc.Bacc("TRN2", target_bir_lowering=False, debug=False,
                   num_devices=N_CORES)
    hT = nc.dram_tensor("hT", [H, S], BF16, kind="ExternalInput")
    wq = nc.dram_tensor("wq", [H, HC], BF16, kind="ExternalInput")
    wk = nc.dram_tensor("wk", [H, HC], BF16, kind="ExternalInput")
    wv = nc.dram_tensor("wv", [H, HC], BF16, kind="ExternalInput")
    wo = nc.dram_tensor("wo", [HC, H], BF16, kind="ExternalInput")
    cosT = nc.dram_tensor("cosT", [D, S], F32, kind="ExternalInput")
    sinTs = nc.dram_tensor("sinTs", [D, S], F32, kind="ExternalInput")
    if causal:
        mtri = nc.dram_tensor("mtri", [128, CH], F32, kind="ExternalInput")
    else:
        maskT = nc.dram_tensor("maskT", [S, S], F32, kind="ExternalInput")
    po = nc.dram_tensor("po", [H, S], BF16, kind="ExternalOutput")

    def mm(out, lhsT, rhs, start, stop, skip=False):
        nc.tensor.matmul(out, lhsT, rhs, start=start, stop=stop,
                         skip_group_check=skip)

    ts = bass.ts

    with tile.TileContext(nc) as tc:
        with tc.tile_pool(name="wkp", bufs=KT_TILES) as wkp, \
             tc.tile_pool(name="wqp", bufs=KT_TILES) as wqp, \
             tc.tile_pool(name="wvp", bufs=KT_TILES) as wvp, \
             tc.tile_pool(name="wop", bufs=HPC) as wop, \
             tc.tile_pool(name="ktp", bufs=HPC) as ktp, \
             tc.tile_pool(name="vpool", bufs=SJT) as vpool, \
             tc.tile_pool(name="csp", bufs=2) as csp, \
             tc.tile_pool(name="htp", bufs=7) as htp, \
             tc.tile_pool(name="qtp", bufs=2 * HPC) as qtp, \
             tc.tile_pool(name="ropep", bufs=2) as ropep, \
             tc.tile_pool(name="esp", bufs=2) as esp, \
             tc.tile_pool(name="accp", bufs=4) as accp, \
             tc.tile_pool(name="attp", bufs=8) as attp, \
             tc.tile_pool(name="potp", bufs=2) as potp, \
             tc.tile_pool(name="mp", bufs=3) as mp, \
             tc.tile_pool(name="pp", bufs=8, space="PSUM") as pp:

            WK = [wkp.tile([128, HC], BF16, tag="wk", name=f"WK{k}")
                  for k in range(KT_TILES)]
            WQ = [wqp.tile([128, HC], BF16, tag="wq", name=f"WQ{k}")
                  for k in range(KT_TILES)]
            WV = [wvp.tile([128, HC], BF16, tag="wv", name=f"WV{k}")
                  for k in range(KT_TILES)]
            WO = [wop.tile([128, H], BF16, tag="wo", name=f"WO{k}")
                  for k in range(HPC)]
            KT = [ktp.tile([128, S], BF16, tag="kt", name=f"KT{h}")
                  for h in range(HPC)]
            V = [vpool.tile([128, HC], BF16, tag="v", name=f"V{j}")
                 for j in range(SJT)]
            if causal:
                mtri_t = mp.tile([128, CH], F32, tag="mtri", bufs=1,
                                 name="mtri_t")
                nc.sync.dma_start(out=mtri_t[:], in_=mtri[:, :])

            ones_col32 = mp.tile([128, 1], F32, tag="oc32", bufs=1,
                                 name="ones_col32")
            ones_row32 = mp.tile([1, 128], F32, tag="or32", bufs=1,
                                 name="ones_row32")
            nc.vector.memset(ones_col32[:], 1.0)
            nc.vector.memset(ones_row32[:], 1.0)
            ones_col = mp.tile([128, 1], F32R, tag="oc", bufs=1,
                              name="ones_col")
            ones_row = mp.tile([1, 128], F32R, tag="or", bufs=1,
                              name="ones_row")
            ones_col_bf = mp.tile([128, 1], BF16, tag="ocb", bufs=1,
                                  name="ones_col_bf")
            nc.vector.tensor_copy(ones_col[:], ones_col32[:])
            nc.vector.tensor_copy(ones_row[:], ones_row32[:])
            nc.vector.tensor_copy(ones_col_bf[:], ones_col32[:])

            for kl in range(HPC):
                nc.sync.dma_start(out=WO[kl][:], in_=wo[ts(kl, 128), :])

            attT_map = {}
            QTc_map = {}
            cs_map = {}
            # rolling ht prefetch stream across all passes and chunks:
            # global step g in [0, NCH*96): chunk g//96, k-tile g%32
            ht_stream = {}
            HT_LEAD = 5
            HT_STEPS = NCH * 3 * KT_TILES

            def ht_fetch(g):
                for gg in range(g, min(g + HT_LEAD + 1, HT_STEPS)):
                    if gg not in ht_stream:
                        cc = gg // (3 * KT_TILES)
                        kk = gg % KT_TILES
                        t = htp.tile([128, CH], BF16, tag="ht", bufs=7,
                                     name="ht")
                        nc.sync.dma_start(
                            out=t[:], in_=hT[ts(kk, 128), ts(cc, CH)])
                        ht_stream[gg] = t
                return ht_stream.pop(g)

            def rope(ps, dst_ap, c):
                cosc, sinc = cs_map[c]
                raw = ropep.tile([128, CH], F32, tag="raw", bufs=1,
                                 name="raw")
                nc.vector.tensor_copy(raw[:], ps[:])
                shf = ropep.tile([128, CH], F32, tag="shf", bufs=1,
                                 name="shf")
                nc.sync.dma_start(out=shf[0:64, :], in_=raw[64:128, :])
                nc.sync.dma_start(out=shf[64:128, :], in_=raw[0:64, :])
                tmp = ropep.tile([128, CH], F32, tag="rtmp", bufs=1,
                                 name="rtmp")
                nc.vector.tensor_mul(tmp[:], shf[:], sinc[:])
                nc.vector.tensor_mul(dst_ap, raw[:], cosc[:])
                nc.vector.tensor_add(dst_ap, dst_ap, tmp[:])

            # ---------------- pass emitters for one chunk ----------------
            def make_pass_emitters(c):
                ems = []
                kps = [None] * HPC
                qps = [None] * HPC
                vps = [None] * HPC

                def cs_load():
                    cosc = csp.tile([128, CH], F32, tag="cos", bufs=1,
                                    name="cosc")
                    sinc = csp.tile([128, CH], F32, tag="sin", bufs=1,
                                    name="sinc")
                    nc.sync.dma_start(out=cosc[:], in_=cosT[:, ts(c, CH)])
                    nc.sync.dma_start(out=sinc[:], in_=sinTs[:, ts(c, CH)])
                    cs_map[c] = (cosc, sinc)
                ems.append(cs_load)

                def kstep(k, W, wsrc, acc, out_of, passi):
                    def emit():
                        if c == 0 and wsrc is not None:
                            nc.sync.dma_start(out=W[k][:],
                                              in_=wsrc[ts(k, 128), :])
                        g = (c * 3 + passi) * KT_TILES + k
                        ht = ht_fetch(g)
                        st_, sp_ = (k == 0), (k == KT_TILES - 1)
                        if k == 0:
                            for d in range(HPC):
                                acc[d] = pp.tile([128, CH], F32,
                                                 tag="acc", bufs=4,
                                                 name=f"{out_of}{d}")
                        if out_of == "vps":
                            for jl in range(HPC):
                                mm(acc[jl][:], ht[:, ts(jl, 128)],
                                   W[k][:], st_, sp_)
                        else:
                            for d in range(HPC):
                                mm(acc[d][:], W[k][:, ts(d, 128)],
                                   ht[:], st_, sp_)
                    return emit

                # K pass + rope-K
                for k in range(KT_TILES):
                    ems.append(kstep(k, WK, wk, kps, "kps", 0))

                def ropek(d):
                    def emit():
                        rope(kps[d], KT[d][:, ts(c, CH)], c)
                    return emit
                for d in range(HPC):
                    ems.append(ropek(d))

                # Q pass + rope-Q
                for k in range(KT_TILES):
                    ems.append(kstep(k, WQ, wq, qps, "qps", 1))

                def ropeq(d):
                    def emit():
                        if d == 0:
                            QTc_map[c] = [
                                qtp.tile([128, CH], BF16, tag="qtc",
                                         bufs=2 * HPC, name=f"QTc{i}")
                                for i in range(HPC)]
                        rope(qps[d], QTc_map[c][d][:], c)
                    return emit
                for d in range(HPC):
                    ems.append(ropeq(d))

                # V pass + evict
                for k in range(KT_TILES):
                    ems.append(kstep(k, WV, wv, vps, "vps", 2))

                def vevict(jl):
                    def emit():
                        nc.scalar.copy(out=V[4 * c + jl][:],
                                       in_=vps[jl][:])
                    return emit
                for jl in range(HPC):
                    ems.append(vevict(jl))
                return ems

            # ---------------- attention emitters for one chunk ----------
            def make_head_emitters(c, h):
                jmax = 4 * c + 4 if causal else SJT
                st = {}

                def emit_pv(j):
                    p = j - 4 * c if causal else -1
                    if p >= 1:
                        mm(st['o'][:, p * 128:], V[j][:, ts(h, 128)],
                           st['es'][j][:, p * 128:],
                           False, j == jmax - 1, skip=True)
                    else:
                        mm(st['o'][:], V[j][:, ts(h, 128)],
                           st['es'][j][:], j == 0, j == jmax - 1,
                           skip=True)

                def block(j):
                    def emit():
                        if j == 0:
                            st['acc'] = accp.tile([128, CH], F32R,
                                                  tag="esacc", bufs=1,
                                                  name="esacc")
                            st['o'] = pp.tile([128, CH], F32, tag="ov",
                                              bufs=1, name="o_ps")
                            st['es'] = {}
                        s = pp.tile([128, CH], F32, tag="sc", bufs=2,
                                    name="s_ps")
                        es = esp.tile([128, CH], BF16, tag="es", bufs=2,
                                      name="es")
                        p = j - 4 * c if causal else -1
                        if causal and p >= 1:
                            # staircase: only columns [p*128, 512) are live
                            mm(s[:, p * 128:], KT[h][:, ts(j, 128)],
                               QTc_map[c][h][:, p * 128:], True, True)
                            nc.gpsimd.memset(es[:, 0:p * 128], 0.0)
                        else:
                            mm(s[:], KT[h][:, ts(j, 128)],
                               QTc_map[c][h][:], True, True)
                        if causal and p >= 0:
                            nc.vector.tensor_add(
                                s[:, ts(p, 128)], s[:, ts(p, 128)],
                                mtri_t[:, ts(p, 128)])
                            nc.scalar.activation(es[:, p * 128:],
                                                 s[:, p * 128:], EXPF)
                        else:
                            if not causal:
                                mg = mp.tile([128, CH], F32, tag="mg",
                                             bufs=3, name="mg")
                                nc.sync.dma_start(
                                    out=mg[:],
                                    in_=maskT[ts(j, 128), ts(c, CH)])
                                nc.vector.tensor_add(s[:], s[:], mg[:])
                            nc.scalar.activation(es[:], s[:], EXPF)
                        if j == 0:
                            nc.gpsimd.tensor_copy(st['acc'][:], es[:])
                        elif j < jmax - 1:
                            # last block's es feeds the sums matmul directly
                            nc.gpsimd.tensor_add(st['acc'][:],
                                                 st['acc'][:], es[:])
                        st['es'][j] = es
                        if j > 0:
                            emit_pv(j - 1)
                    return emit

                def tail1():
                    emit_pv(jmax - 1)
                    sums = pp.tile([128, CH], F32, tag="sc", bufs=2,
                                   name="sums")
                    mm(sums[0:1, :], ones_col[:], st['acc'][:],
                       True, False)
                    mm(sums[0:1, :], ones_col_bf[:],
                       st['es'][jmax - 1][:], False, True)
                    ssb = accp.tile([1, CH], F32R, tag="ssb", bufs=1,
                                    name="ssb")
                    nc.scalar.copy(out=ssb[:], in_=sums[0:1, :])
                    st['ssb'] = ssb

                def tail2():
                    # broadcast raw sums to 128 partitions, then take the
                    # reciprocal on DVE (off the PE dependency chain)
                    b_ps = pp.tile([128, CH], F32, tag="sc", bufs=2,
                                   name="b_ps")
                    mm(b_ps[:], ones_row[:], st['ssb'][:], True, True)
                    rb = accp.tile([128, CH], F32R, tag="rb", bufs=1,
                                   name="rb")
                    with nc.allow_low_precision(reason="softmax recip"):
                        nc.vector.reciprocal(rb[:], b_ps[:])
                    att = attp.tile([128, CH], BF16, tag="attT", bufs=8,
                                    name="att")
                    nc.vector.tensor_mul(att[:], st['o'][:], rb[:])
                    attT_map[(c, h)] = att

                return [block(j) for j in range(jmax)] + [tail1], tail2

            def make_attn_emitters(c):
                # weave each head's tail2 after the NEXT head's first block
                # (but before its second, which allocates/needs o_ps) so the
                # reciprocal chain latency is covered by PE work.
                ems = []
                carry = None
                for h in range(HPC):
                    head, t2 = make_head_emitters(c, h)
                    ems.append(head[0])
                    if carry is not None:
                        ems.append(carry)
                    ems += head[1:]
                    carry = t2
                ems.append(carry)
                return ems

            # ---------------- o_proj emitters for one chunk -------------
            def make_oproj_emitters(cc, tag):
                ats = [attT_map[(cc, h)] for h in range(HPC)]
                bufs = 4 if tag == "acc" else 1

                def otile(n):
                    def emit():
                        pps = pp.tile([128, CH], F32, tag=tag, bufs=bufs,
                                      name="pps")
                        for kl in range(HPC):
                            mm(pps[:], WO[kl][:, ts(n, 128)], ats[kl][:],
                               kl == 0, kl == HPC - 1)
                        ot = potp.tile([128, CH], BF16, tag="pot", bufs=2,
                                       name="ot")
                        nc.scalar.copy(out=ot[:], in_=pps[:])
                        nc.sync.dma_start(out=po[ts(n, 128), ts(cc, CH)],
                                          in_=ot[:])
                    return emit
                return [otile(n) for n in range(H // 128)]

            def interleave(spine, fillers):
                fi = 0
                for i, em in enumerate(spine):
                    em()
                    tgt = (i + 1) * len(fillers) // len(spine)
                    while fi < tgt:
                        fillers[fi]()
                        fi += 1
                while fi < len(fillers):
                    fillers[fi]()
                    fi += 1

            # ---------------- software-pipelined schedule ----------------
            oproj_fns = {}
            for c in range(NCH):
                fill = []
                if c >= 1:
                    fill = make_attn_emitters(c - 1)
                if c >= 2:
                    fill = _fair_merge(fill,
                                       make_oproj_emitters(c - 2, "pps"))
                interleave(make_pass_emitters(c), fill)

            # tail: attention(3) x o_proj(2), then o_proj(3)
            interleave(make_attn_emitters(NCH - 1),
                       make_oproj_emitters(NCH - 2, "acc"))
            for em in make_oproj_emitters(NCH - 1, "acc"):
                em()

    nc.compile()
    return nc


_CACHE = {}


def _get_nc(causal):
    if causal not in _CACHE:
        _CACHE[causal] = _build(causal)
    return _CACHE[causal]


def kernel(hidden_states, attention_mask, position_ids, Wq, Wk, Wv, Wo):
    global last_exec_time_ns
    B, S_, H_ = hidden_states.shape
    assert (B, S_, H_) == (1, S, H)
    hs = np.asarray(hidden_states, dtype=np.float32)
    mask = np.asarray(attention_mask, dtype=np.float32)[0, 0]
    pos = np.asarray(position_ids)[0].astype(np.float64)

    iu = np.triu_indices(S, k=1)
    il = np.tril_indices(S, k=0)
    causal = bool(np.all(mask[il] == 0.0) and np.all(mask[iu] <= -1e30))

    hT_b = _b(hs[0].T)
    scale = 1.0 / np.sqrt(D)

    inv_freq = 1.0 / (ROPE_BASE ** (np.arange(0, D, 2, dtype=np.float64) / D))
    ang = pos[None, :] * np.concatenate([inv_freq, inv_freq])[:, None]  # [D,S]
    cosT = _r(np.cos(ang))
    sgn = np.ones((D, 1)); sgn[: D // 2] = -1.0
    sinTs = _r(np.sin(ang) * sgn)

    if causal:
        # 4 diagonal-block triangle patterns packed into [128, 512]:
        # pattern p in cols [128p, 128p+128), NEG where key-row r > query-col
        mtri = np.zeros((128, CH), dtype=np.float32)
        rr = np.arange(128)[:, None]
        qq = np.arange(128)[None, :]
        for p in range(4):
            blk = mtri[:, p * 128:(p + 1) * 128]
            blk[rr > qq] = NEG
        mtri = _r(mtri)
    else:
        maskT = _r(mask.T)

    nc = _get_nc(causal)
    in_maps = []
    for c in range(N_CORES):
        sl = slice(c * HC, (c + 1) * HC)
        m = {
            "hT": hT_b,
            "wq": _b(np.asarray(Wq, np.float64)[:, sl] * scale),
            "wk": _b(np.asarray(Wk)[:, sl]),
            "wv": _b(np.asarray(Wv)[:, sl]),
            "wo": _b(np.asarray(Wo)[sl, :]),
            "cosT": cosT,
            "sinTs": sinTs,
        }
        if causal:
            m["mtri"] = mtri
        else:
            m["maskT"] = maskT
        in_maps.append(m)

    trace = bool(int(os.environ.get("BASS_KERNEL_TRACE", "0")))
    kw = {}
    if trace:
        kw["trace"] = True
        kw["tmpdir"] = os.environ.get("BASS_KERNEL_TRACE_DIR") or None
    res = run_bass_kernel_spmd(nc, in_maps, list(range(N_CORES)), **kw)
    last_exec_time_ns = res.exec_time_ns

    acc = np.zeros((H, S), dtype=np.float32)
    for c in range(N_CORES):
        acc += np.asarray(res.results[c]["po"], dtype=np.float32)
    out = acc.T.reshape(1, S, H)
    return out


# revision 24
# speedup vs baseline: 1.0391x; 1.0391x over previous
"""Trainium2 Bass kernel for LlamaAttention (B=1, S=2048, H=4096, 32 heads).

Tensor-parallel over heads: 8 cores x 4 heads. v4 design:
  - All matmuls bf16 (1 cycle/row, ldweights ~99ns fully hidden),
    accumulation in fp32 PSUM.
  - Wq/Wk/Wv/Wo slices resident in SBUF, loaded once during chunk 0.
  - Software pipeline across 512-wide seq chunks:
      chunk c projection passes (K -> rope-K, Q -> rope-Q, V) are emitted
      interleaved with attention(c-1) blocks and o_proj(c-2) tiles, so the
      tensor engine always has dense matmul work while exp (ACT), softmax
      sums (DVE) and rope (ACT+DMA+DVE) latencies play out on other engines.
      Tail: attention(3) x o_proj(2), then o_proj(3) on the freed acc ring.
  - RoPE rotate-half: ACT evict to SBUF + DMA partition shift + DVE combine.
  - Softmax: es accumulated on DVE, partition-reduced by a ones-matmul,
    1/sum broadcast by a rank-1 matmul (both inside the sc PSUM ring).
  - Causal staircase: diagonal blocks only mask/exp the valid columns.
  - po partials written bf16; host sums the 8 partials in fp32.
PSUM banks: acc=4 (kps/qps/vps + tail o_proj), sc=2, ov=1, pps=1.
"""

import os
import sys

if "/opt/trn_rl_repo" not in sys.path:
    sys.path.insert(0, "/opt/trn_rl_repo")

import numpy as np
import ml_dtypes

from concourse import bacc, mybir, tile
from concourse import bass
from concourse.bass_utils import run_bass_kernel_spmd

F32 = mybir.dt.float32
F32R = mybir.dt.float32r
BF16 = mybir.dt.bfloat16
EXPF = mybir.ActivationFunctionType.Exp

N_CORES = 8
S = 2048
H = 4096
N_HEADS = 32
D = 128
HPC = N_HEADS // N_CORES     # 4 heads per core
HC = HPC * D                 # 512
CH = 512
NCH = S // CH                # 4
KT_TILES = H // 128          # 32
SJT = S // 128               # 16
ROPE_BASE = 10000.0
NEG = -1.0e9

last_exec_time_ns = None

BFDT = ml_dtypes.bfloat16


def _r(x):
    return np.ascontiguousarray(x, dtype=np.float32)


def _b(x):
    return np.ascontiguousarray(np.asarray(x, dtype=np.float32), dtype=BFDT)


def _fair_merge(a, b):
    """Round-robin merge two emitter lists proportionally."""
    out = []
    na, nb = len(a), len(b)
    ia = ib = 0
    for _ in range(na + nb):
        if ib * na <= ia * nb and ib < nb:
            out.append(b[ib]); ib += 1
        elif ia < na:
            out.append(a[ia]); ia += 1
        else:
            out.append(b[ib]); ib += 1
    return out


def _build(causal: bool):
    nc = bacc.Bacc("TRN2", target_bir_lowering=False, debug=False,
                   num_devices=N_CORES)
    hT = nc.dram_tensor("hT", [H, S], BF16, kind="ExternalInput")
    wq = nc.dram_tensor("wq", [H, HC], BF16, kind="ExternalInput")
    wk = nc.dram_tensor("wk", [H, HC], BF16, kind="ExternalInput")
    wv = nc.dram_tensor("wv", [H, HC], BF16, kind="ExternalInput")
    wo = nc.dram_tensor("wo", [HC, H], BF16, kind="ExternalInput")
    cosT = nc.dram_tensor("cosT", [D, S], F32, kind="ExternalInput")
    sinTs = nc.dram_tensor("sinTs", [D, S], F32, kind="ExternalInput")
    if causal:
        mtri = nc.dram_tensor("mtri", [128, CH], F32, kind="ExternalInput")
    else:
        maskT = nc.dram_tensor("maskT", [S, S], F32, kind="ExternalInput")
    po = nc.dram_tensor("po", [H, S], BF16, kind="ExternalOutput")

    def mm(out, lhsT, rhs, start, stop, skip=False):
        nc.tensor.matmul(out, lhsT, rhs, start=start, stop=stop,
                         skip_group_check=skip)

    ts = bass.ts

    with tile.TileContext(nc) as tc:
        with tc.tile_pool(name="wkp", bufs=KT_TILES) as wkp, \
             tc.tile_pool(name="wqp", bufs=KT_TILES) as wqp, \
             tc.tile_pool(name="wvp", bufs=KT_TILES) as wvp, \
             tc.tile_pool(name="wop", bufs=HPC) as wop, \
             tc.tile_pool(name="ktp", bufs=HPC) as ktp, \
             tc.tile_pool(name="vpool", bufs=SJT) as vpool, \
             tc.tile_pool(name="csp", bufs=2) as csp, \
             tc.tile_pool(name="htp", bufs=7) as htp, \
             tc.tile_pool(name="qtp", bufs=2 * HPC) as qtp, \
             tc.tile_pool(name="ropep", bufs=2) as ropep, \
             tc.tile_pool(name="esp", bufs=2) as esp, \
             tc.tile_pool(name="accp", bufs=4) as accp, \
             tc.tile_pool(name="attp", bufs=8) as attp, \
             tc.tile_pool(name="potp", bufs=2) as potp, \
             tc.tile_pool(name="mp", bufs=3) as mp, \
             tc.tile_pool(name="pp", bufs=8, space="PSUM") as pp:

            WK = [wkp.tile([128, HC], BF16, tag="wk", name=f"WK{k}")
                  for k in range(KT_TILES)]
            WQ = [wqp.tile([128, HC], BF16, tag="wq", name=f"WQ{k}")
                  for k in range(KT_TILES)]
            WV = [wvp.tile([128, HC], BF16, tag="wv", name=f"WV{k}")
                  for k in range(KT_TILES)]
            WO = [wop.tile([128, H], BF16, tag="wo", name=f"WO{k}")
                  for k in range(HPC)]
            KT = [ktp.tile([128, S], BF16, tag="kt", name=f"KT{h}")
                  for h in range(HPC)]
            V = [vpool.tile([128, HC], BF16, tag="v", name=f"V{j}")
                 for j in range(SJT)]
            if causal:
                mtri_t = mp.tile([128, CH], F32, tag="mtri", bufs=1,
                                 name="mtri_t")
                nc.sync.dma_start(out=mtri_t[:], in_=mtri[:, :])

            ones_col32 = mp.tile([128, 1], F32, tag="oc32", bufs=1,
                                 name="ones_col32")
            ones_row32 = mp.tile([1, 128], F32, tag="or32", bufs=1,
                                 name="ones_row32")
            nc.vector.memset(ones_col32[:], 1.0)
            nc.vector.memset(ones_row32[:], 1.0)
            ones_col = mp.tile([128, 1], F32R, tag="oc", bufs=1,
                              name="ones_col")
            ones_row = mp.tile([1, 128], F32R, tag="or", bufs=1,
                              name="ones_row")
            ones_col_bf = mp.tile([128, 1], BF16, tag="ocb", bufs=1,
                                  name="ones_col_bf")
            nc.vector.tensor_copy(ones_col[:], ones_col32[:])
            nc.vector.tensor_copy(ones_row[:], ones_row32[:])
            nc.vector.tensor_copy(ones_col_bf[:], ones_col32[:])

            for kl in range(HPC):
                nc.sync.dma_start(out=WO[kl][:], in_=wo[ts(kl, 128), :])

            attT_map = {}
            QTc_map = {}
            cs_map = {}
            # rolling ht prefetch stream across all passes and chunks:
            # global step g in [0, NCH*96): chunk g//96, k-tile g%32
            ht_stream = {}
            HT_LEAD = 5
            HT_STEPS = NCH * 3 * KT_TILES

            def ht_fetch(g):
                for gg in range(g, min(g + HT_LEAD + 1, HT_STEPS)):
                    if gg not in ht_stream:
                        cc = gg // (3 * KT_TILES)
                        kk = gg % KT_TILES
                        t = htp.tile([128, CH], BF16, tag="ht", bufs=7,
                                     name="ht")
                        nc.sync.dma_start(
                            out=t[:], in_=hT[ts(kk, 128), ts(cc, CH)])
                        ht_stream[gg] = t
                return ht_stream.pop(g)

            def rope(ps, dst_ap, c):
                cosc, sinc = cs_map[c]
                raw = ropep.tile([128, CH], F32, tag="raw", bufs=1,
                                 name="raw")
                nc.scalar.copy(out=raw[:], in_=ps[:])
                shf = ropep.tile([128, CH], F32, tag="shf", bufs=1,
                                 name="shf")
                nc.sync.dma_start(out=shf[0:64, :], in_=raw[64:128, :])
                nc.sync.dma_start(out=shf[64:128, :], in_=raw[0:64, :])
                tmp = ropep.tile([128, CH], F32, tag="rtmp", bufs=1,
                                 name="rtmp")
                nc.vector.tensor_mul(tmp[:], shf[:], sinc[:])
                nc.vector.tensor_mul(dst_ap, raw[:], cosc[:])
                nc.vector.tensor_add(dst_ap, dst_ap, tmp[:])

            # ---------------- pass emitters for one chunk ----------------
            def make_pass_emitters(c):
                ems = []
                kps = [None] * HPC
                qps = [None] * HPC
                vps = [None] * HPC

                def cs_load():
                    cosc = csp.tile([128, CH], F32, tag="cos", bufs=1,
                                    name="cosc")
                    sinc = csp.tile([128, CH], F32, tag="sin", bufs=1,
                                    name="sinc")
                    nc.sync.dma_start(out=cosc[:], in_=cosT[:, ts(c, CH)])
                    nc.sync.dma_start(out=sinc[:], in_=sinTs[:, ts(c, CH)])
                    cs_map[c] = (cosc, sinc)
                ems.append(cs_load)

                def kstep(k, W, wsrc, acc, out_of, passi):
                    def emit():
                        if c == 0 and wsrc is not None:
                            nc.sync.dma_start(out=W[k][:],
                                              in_=wsrc[ts(k, 128), :])
                        g = (c * 3 + passi) * KT_TILES + k
                        ht = ht_fetch(g)
                        st_, sp_ = (k == 0), (k == KT_TILES - 1)
                        if k == 0:
                            for d in range(HPC):
                                acc[d] = pp.tile([128, CH], F32,
                                                 tag="acc", bufs=4,
                                                 name=f"{out_of}{d}")
                        if out_of == "vps":
                            for jl in range(HPC):
                                mm(acc[jl][:], ht[:, ts(jl, 128)],
                                   W[k][:], st_, sp_)
                        else:
                            for d in range(HPC):
                                mm(acc[d][:], W[k][:, ts(d, 128)],
                                   ht[:], st_, sp_)
                    return emit

                # K pass + rope-K
                for k in range(KT_TILES):
                    ems.append(kstep(k, WK, wk, kps, "kps", 0))

                def ropek(d):
                    def emit():
                        rope(kps[d], KT[d][:, ts(c, CH)], c)
                    return emit
                for d in range(HPC):
                    ems.append(ropek(d))

                # Q pass + rope-Q
                for k in range(KT_TILES):
                    ems.append(kstep(k, WQ, wq, qps, "qps", 1))

                def ropeq(d):
                    def emit():
                        if d == 0:
                            QTc_map[c] = [
                                qtp.tile([128, CH], BF16, tag="qtc",
                                         bufs=2 * HPC, name=f"QTc{i}")
                                for i in range(HPC)]
                        rope(qps[d], QTc_map[c][d][:], c)
                    return emit
                for d in range(HPC):
                    ems.append(ropeq(d))

                # V pass + evict
                for k in range(KT_TILES):
                    ems.append(kstep(k, WV, wv, vps, "vps", 2))

                def vevict(jl):
                    def emit():
                        nc.scalar.copy(out=V[4 * c + jl][:],
                                       in_=vps[jl][:])
                    return emit
                for jl in range(HPC):
                    ems.append(vevict(jl))
                return ems

            # ---------------- attention emitters for one chunk ----------
            def make_head_emitters(c, h):
                jmax = 4 * c + 4 if causal else SJT
                st = {}

                def emit_pv(j):
                    p = j - 4 * c if causal else -1
                    if p >= 1:
                        mm(st['o'][:, p * 128:], V[j][:, ts(h, 128)],
                           st['es'][j][:, p * 128:],
                           False, j == jmax - 1, skip=True)
                    else:
                        mm(st['o'][:], V[j][:, ts(h, 128)],
                           st['es'][j][:], j == 0, j == jmax - 1,
                           skip=True)

                def block(j):
                    def emit():
                        if j == 0:
                            st['acc'] = accp.tile([128, CH], F32R,
                                                  tag="esacc", bufs=1,
                                                  name="esacc")
                            st['o'] = pp.tile([128, CH], F32, tag="ov",
                                              bufs=1, name="o_ps")
                            st['es'] = {}
                        s = pp.tile([128, CH], F32, tag="sc", bufs=2,
                                    name="s_ps")
                        es = esp.tile([128, CH], BF16, tag="es", bufs=2,
                                      name="es")
                        p = j - 4 * c if causal else -1
                        if causal and p >= 1:
                            # staircase: only columns [p*128, 512) are live
                            mm(s[:, p * 128:], KT[h][:, ts(j, 128)],
                               QTc_map[c][h][:, p * 128:], True, True)
                            nc.gpsimd.memset(es[:, 0:p * 128], 0.0)
                        else:
                            mm(s[:], KT[h][:, ts(j, 128)],
                               QTc_map[c][h][:], True, True)
                        if causal and p >= 0:
                            nc.vector.tensor_add(
                                s[:, ts(p, 128)], s[:, ts(p, 128)],
                                mtri_t[:, ts(p, 128)])
                            nc.scalar.activation(es[:, p * 128:],
                                                 s[:, p * 128:], EXPF)
                        else:
                            if not causal:
                                mg = mp.tile([128, CH], F32, tag="mg",
                                             bufs=3, name="mg")
                                nc.sync.dma_start(
                                    out=mg[:],
                                    in_=maskT[ts(j, 128), ts(c, CH)])
                                nc.vector.tensor_add(s[:], s[:], mg[:])
                            nc.scalar.activation(es[:], s[:], EXPF)
                        if j == 0:
                            nc.vector.tensor_copy(st['acc'][:], es[:])
                        elif j < jmax - 1:
                            # last block's es feeds the sums matmul directly
                            nc.vector.tensor_add(st['acc'][:],
                                                 st['acc'][:], es[:])
                        st['es'][j] = es
                        if j > 0:
                            emit_pv(j - 1)
                    return emit

                def tail1():
                    emit_pv(jmax - 1)
                    sums = pp.tile([128, CH], F32, tag="sc", bufs=2,
                                   name="sums")
                    mm(sums[0:1, :], ones_col[:], st['acc'][:],
                       True, False)
                    mm(sums[0:1, :], ones_col_bf[:],
                       st['es'][jmax - 1][:], False, True)
                    ssb = accp.tile([1, CH], F32R, tag="ssb", bufs=1,
                                    name="ssb")
                    nc.scalar.copy(out=ssb[:], in_=sums[0:1, :])
                    st['ssb'] = ssb

                def tail2():
                    # broadcast raw sums to 128 partitions, then take the
                    # reciprocal on DVE (off the PE dependency chain)
                    b_ps = pp.tile([128, CH], F32, tag="sc", bufs=2,
                                   name="b_ps")
                    mm(b_ps[:], ones_row[:], st['ssb'][:], True, True)
                    rb = accp.tile([128, CH], F32R, tag="rb", bufs=1,
                                   name="rb")
                    with nc.allow_low_precision(reason="softmax recip"):
                        nc.vector.reciprocal(rb[:], b_ps[:])
                    att = attp.tile([128, CH], BF16, tag="attT", bufs=8,
                                    name="att")
                    nc.vector.tensor_mul(att[:], st['o'][:], rb[:])
                    attT_map[(c, h)] = att

                return [block(j) for j in range(jmax)] + [tail1], tail2

            def make_attn_emitters(c):
                # weave each head's tail2 after the NEXT head's first block
                # (but before its second, which allocates/needs o_ps) so the
                # reciprocal chain latency is covered by PE work.
                ems = []
                carry = None
                for h in range(HPC):
                    head, t2 = make_head_emitters(c, h)
                    ems.append(head[0])
                    if carry is not None:
                        ems.append(carry)
                    ems += head[1:]
                    carry = t2
                ems.append(carry)
                return ems

            # ---------------- o_proj emitters for one chunk -------------
            def make_oproj_emitters(cc, tag):
                ats = [attT_map[(cc, h)] for h in range(HPC)]
                bufs = 4 if tag == "acc" else 1

                def otile(n):
                    def emit():
                        pps = pp.tile([128, CH], F32, tag=tag, bufs=bufs,
                                      name="pps")
                        for kl in range(HPC):
                            mm(pps[:], WO[kl][:, ts(n, 128)], ats[kl][:],
                               kl == 0, kl == HPC - 1)
                        ot = potp.tile([128, CH], BF16, tag="pot", bufs=2,
                                       name="ot")
                        if n % 2 == 0:
                            nc.scalar.copy(out=ot[:], in_=pps[:])
                        else:
                            nc.vector.tensor_copy(ot[:], pps[:])
                        nc.sync.dma_start(out=po[ts(n, 128), ts(cc, CH)],
                                          in_=ot[:])
                    return emit
                return [otile(n) for n in range(H // 128)]

            def interleave(spine, fillers):
                fi = 0
                for i, em in enumerate(spine):
                    em()
                    tgt = (i + 1) * len(fillers) // len(spine)
                    while fi < tgt:
                        fillers[fi]()
                        fi += 1
                while fi < len(fillers):
                    fillers[fi]()
                    fi += 1

            # ---------------- software-pipelined schedule ----------------
            oproj_fns = {}
            for c in range(NCH):
                fill = []
                if c >= 1:
                    fill = make_attn_emitters(c - 1)
                if c >= 2:
                    fill = _fair_merge(fill,
                                       make_oproj_emitters(c - 2, "pps"))
                interleave(make_pass_emitters(c), fill)

            # tail: attention(3) x o_proj(2), then o_proj(3)
            interleave(make_attn_emitters(NCH - 1),
                       make_oproj_emitters(NCH - 2, "acc"))
            for em in make_oproj_emitters(NCH - 1, "acc"):
                em()

    nc.compile()
    return nc


_CACHE = {}


def _get_nc(causal):
    if causal not in _CACHE:
        _CACHE[causal] = _build(causal)
    return _CACHE[causal]


def kernel(hidden_states, attention_mask, position_ids, Wq, Wk, Wv, Wo):
    global last_exec_time_ns
    B, S_, H_ = hidden_states.shape
    assert (B, S_, H_) == (1, S, H)
    hs = np.asarray(hidden_states, dtype=np.float32)
    mask = np.asarray(attention_mask, dtype=np.float32)[0, 0]
    pos = np.asarray(position_ids)[0].astype(np.float64)

    iu = np.triu_indices(S, k=1)
    il = np.tril_indices(S, k=0)
    causal = bool(np.all(mask[il] == 0.0) and np.all(mask[iu] <= -1e30))

    hT_b = _b(hs[0].T)
    scale = 1.0 / np.sqrt(D)

    inv_freq = 1.0 / (ROPE_BASE ** (np.arange(0, D, 2, dtype=np.float64) / D))
    ang = pos[None, :] * np.concatenate([inv_freq, inv_freq])[:, None]  # [D,S]
    cosT = _r(np.cos(ang))
    sgn = np.ones((D, 1)); sgn[: D // 2] = -1.0
    sinTs = _r(np.sin(ang) * sgn)

    if causal:
        # 4 diagonal-block triangle patterns packed into [128, 512]:
        # pattern p in cols [128p, 128p+128), NEG where key-row r > query-col
        mtri = np.zeros((128, CH), dtype=np.float32)
        rr = np.arange(128)[:, None]
        qq = np.arange(128)[None, :]
        for p in range(4):
            blk = mtri[:, p * 128:(p + 1) * 128]
            blk[rr > qq] = NEG
        mtri = _r(mtri)
    else:
        maskT = _r(mask.T)

    nc = _get_nc(causal)
    in_maps = []
    for c in range(N_CORES):
        sl = slice(c * HC, (c + 1) * HC)
        m = {
            "hT": hT_b,
            "wq": _b(np.asarray(Wq, np.float64)[:, sl] * scale),
            "wk": _b(np.asarray(Wk)[:, sl]),
            "wv": _b(np.asarray(Wv)[:, sl]),
            "wo": _b(np.asarray(Wo)[sl, :]),
            "cosT": cosT,
            "sinTs": sinTs,
        }
        if causal:
            m["mtri"] = mtri
        else:
            m["maskT"] = maskT
        in_maps.append(m)

    trace = bool(int(os.environ.get("BASS_KERNEL_TRACE", "0")))
    kw = {}
    if trace:
        kw["trace"] = True
        kw["tmpdir"] = os.environ.get("BASS_KERNEL_TRACE_DIR") or None
    res = run_bass_kernel_spmd(nc, in_maps, list(range(N_CORES)), **kw)
    last_exec_time_ns = res.exec_time_ns

    acc = np.zeros((H, S), dtype=np.float32)
    for c in range(N_CORES):
        acc += np.asarray(res.results[c]["po"], dtype=np.float32)
    out = acc.T.reshape(1, S, H)
    return out


# revision 25
# speedup vs baseline: 1.0594x; 1.0196x over previous
"""Trainium2 Bass kernel for LlamaAttention (B=1, S=2048, H=4096, 32 heads).

Tensor-parallel over heads: 8 cores x 4 heads. v4 design:
  - All matmuls bf16 (1 cycle/row, ldweights ~99ns fully hidden),
    accumulation in fp32 PSUM.
  - Wq/Wk/Wv/Wo slices resident in SBUF, loaded once during chunk 0.
  - Software pipeline across 512-wide seq chunks:
      chunk c projection passes (K -> rope-K, Q -> rope-Q, V) are emitted
      interleaved with attention(c-1) blocks and o_proj(c-2) tiles, so the
      tensor engine always has dense matmul work while exp (ACT), softmax
      sums (DVE) and rope (ACT+DMA+DVE) latencies play out on other engines.
      Tail: attention(3) x o_proj(2), then o_proj(3) on the freed acc ring.
  - RoPE rotate-half: ACT evict to SBUF + DMA partition shift + DVE combine.
  - Softmax: es accumulated on DVE, partition-reduced by a ones-matmul,
    1/sum broadcast by a rank-1 matmul (both inside the sc PSUM ring).
  - Causal staircase: diagonal blocks only mask/exp the valid columns.
  - po partials written bf16; host sums the 8 partials in fp32.
PSUM banks: acc=4 (kps/qps/vps + tail o_proj), sc=2, ov=1, pps=1.
"""

import os
import sys

if "/opt/trn_rl_repo" not in sys.path:
    sys.path.insert(0, "/opt/trn_rl_repo")

import numpy as np
import ml_dtypes

from concourse import bacc, mybir, tile
from concourse import bass
from concourse.bass_utils import run_bass_kernel_spmd

F32 = mybir.dt.float32
F32R = mybir.dt.float32r
BF16 = mybir.dt.bfloat16
EXPF = mybir.ActivationFunctionType.Exp

N_CORES = 8
S = 2048
H = 4096
N_HEADS = 32
D = 128
HPC = N_HEADS // N_CORES     # 4 heads per core
HC = HPC * D                 # 512
CH = 512
NCH = S // CH                # 4
KT_TILES = H // 128          # 32
SJT = S // 128               # 16
ROPE_BASE = 10000.0
NEG = -1.0e9

last_exec_time_ns = None

BFDT = ml_dtypes.bfloat16


def _r(x):
    return np.ascontiguousarray(x, dtype=np.float32)


def _b(x):
    return np.ascontiguousarray(np.asarray(x, dtype=np.float32), dtype=BFDT)


def _fair_merge(a, b):
    """Round-robin merge two emitter lists proportionally."""
    out = []
    na, nb = len(a), len(b)
    ia = ib = 0
    for _ in range(na + nb):
        if ib * na <= ia * nb and ib < nb:
            out.append(b[ib]); ib += 1
        elif ia < na:
            out.append(a[ia]); ia += 1
        else:
            out.append(b[ib]); ib += 1
    return out


def _build(causal: bool):
    nc = bacc.Bacc("TRN2", target_bir_lowering=False, debug=False,
                   num_devices=N_CORES)
    hT = nc.dram_tensor("hT", [H, S], BF16, kind="ExternalInput")
    wq = nc.dram_tensor("wq", [H, HC], BF16, kind="ExternalInput")
    wk = nc.dram_tensor("wk", [H, HC], BF16, kind="ExternalInput")
    wv = nc.dram_tensor("wv", [H, HC], BF16, kind="ExternalInput")
    wo = nc.dram_tensor("wo", [HC, H], BF16, kind="ExternalInput")
    cosT = nc.dram_tensor("cosT", [D, S], F32, kind="ExternalInput")
    sinTs = nc.dram_tensor("sinTs", [D, S], F32, kind="ExternalInput")
    if causal:
        mtri = nc.dram_tensor("mtri", [128, CH], F32, kind="ExternalInput")
    else:
        maskT = nc.dram_tensor("maskT", [S, S], F32, kind="ExternalInput")
    po = nc.dram_tensor("po", [H, S], BF16, kind="ExternalOutput")

    def mm(out, lhsT, rhs, start, stop, skip=False):
        nc.tensor.matmul(out, lhsT, rhs, start=start, stop=stop,
                         skip_group_check=skip)

    ts = bass.ts

    with tile.TileContext(nc) as tc:
        with tc.tile_pool(name="wkp", bufs=KT_TILES) as wkp, \
             tc.tile_pool(name="wqp", bufs=KT_TILES) as wqp, \
             tc.tile_pool(name="wvp", bufs=KT_TILES) as wvp, \
             tc.tile_pool(name="wop", bufs=HPC) as wop, \
             tc.tile_pool(name="ktp", bufs=HPC) as ktp, \
             tc.tile_pool(name="vpool", bufs=SJT) as vpool, \
             tc.tile_pool(name="csp", bufs=2) as csp, \
             tc.tile_pool(name="htp", bufs=7) as htp, \
             tc.tile_pool(name="qtp", bufs=2 * HPC) as qtp, \
             tc.tile_pool(name="ropep", bufs=2) as ropep, \
             tc.tile_pool(name="esp", bufs=2) as esp, \
             tc.tile_pool(name="accp", bufs=4) as accp, \
             tc.tile_pool(name="attp", bufs=8) as attp, \
             tc.tile_pool(name="potp", bufs=2) as potp, \
             tc.tile_pool(name="mp", bufs=3) as mp, \
             tc.tile_pool(name="pp", bufs=8, space="PSUM") as pp:

            WK = [wkp.tile([128, HC], BF16, tag="wk", name=f"WK{k}")
                  for k in range(KT_TILES)]
            WQ = [wqp.tile([128, HC], BF16, tag="wq", name=f"WQ{k}")
                  for k in range(KT_TILES)]
            WV = [wvp.tile([128, HC], BF16, tag="wv", name=f"WV{k}")
                  for k in range(KT_TILES)]
            WO = [wop.tile([128, H], BF16, tag="wo", name=f"WO{k}")
                  for k in range(HPC)]
            KT = [ktp.tile([128, S], BF16, tag="kt", name=f"KT{h}")
                  for h in range(HPC)]
            V = [vpool.tile([128, HC], BF16, tag="v", name=f"V{j}")
                 for j in range(SJT)]
            if causal:
                mtri_t = mp.tile([128, CH], F32, tag="mtri", bufs=1,
                                 name="mtri_t")
                nc.sync.dma_start(out=mtri_t[:], in_=mtri[:, :])

            ones_col32 = mp.tile([128, 1], F32, tag="oc32", bufs=1,
                                 name="ones_col32")
            ones_row32 = mp.tile([1, 128], F32, tag="or32", bufs=1,
                                 name="ones_row32")
            nc.vector.memset(ones_col32[:], 1.0)
            nc.vector.memset(ones_row32[:], 1.0)
            ones_col = mp.tile([128, 1], F32R, tag="oc", bufs=1,
                              name="ones_col")
            ones_row = mp.tile([1, 128], F32R, tag="or", bufs=1,
                              name="ones_row")
            ones_col_bf = mp.tile([128, 1], BF16, tag="ocb", bufs=1,
                                  name="ones_col_bf")
            nc.vector.tensor_copy(ones_col[:], ones_col32[:])
            nc.vector.tensor_copy(ones_row[:], ones_row32[:])
            nc.vector.tensor_copy(ones_col_bf[:], ones_col32[:])

            for kl in range(HPC):
                nc.sync.dma_start(out=WO[kl][:], in_=wo[ts(kl, 128), :])

            attT_map = {}
            QTc_map = {}
            cs_map = {}
            # rolling ht prefetch stream across all passes and chunks:
            # global step g in [0, NCH*96): chunk g//96, k-tile g%32
            ht_stream = {}
            HT_LEAD = 5
            HT_STEPS = NCH * 3 * KT_TILES

            def ht_fetch(g):
                for gg in range(g, min(g + HT_LEAD + 1, HT_STEPS)):
                    if gg not in ht_stream:
                        cc = gg // (3 * KT_TILES)
                        kk = gg % KT_TILES
                        t = htp.tile([128, CH], BF16, tag="ht", bufs=7,
                                     name="ht")
                        nc.sync.dma_start(
                            out=t[:], in_=hT[ts(kk, 128), ts(cc, CH)])
                        ht_stream[gg] = t
                return ht_stream.pop(g)

            def rope(ps, dst_ap, c):
                cosc, sinc = cs_map[c]
                raw = ropep.tile([128, CH], F32, tag="raw", bufs=1,
                                 name="raw")
                nc.scalar.copy(out=raw[:], in_=ps[:])
                shf = ropep.tile([128, CH], F32, tag="shf", bufs=1,
                                 name="shf")
                nc.sync.dma_start(out=shf[0:64, :], in_=raw[64:128, :])
                nc.sync.dma_start(out=shf[64:128, :], in_=raw[0:64, :])
                tmp = ropep.tile([128, CH], F32, tag="rtmp", bufs=1,
                                 name="rtmp")
                nc.vector.tensor_mul(tmp[:], shf[:], sinc[:])
                nc.vector.tensor_mul(dst_ap, raw[:], cosc[:])
                nc.vector.tensor_add(dst_ap, dst_ap, tmp[:])

            # ---------------- pass emitters for one chunk ----------------
            def make_pass_emitters(c):
                ems = []
                kps = [None] * HPC
                qps = [None] * HPC
                vps = [None] * HPC

                def cs_load():
                    cosc = csp.tile([128, CH], F32, tag="cos", bufs=1,
                                    name="cosc")
                    sinc = csp.tile([128, CH], F32, tag="sin", bufs=1,
                                    name="sinc")
                    nc.sync.dma_start(out=cosc[:], in_=cosT[:, ts(c, CH)])
                    nc.sync.dma_start(out=sinc[:], in_=sinTs[:, ts(c, CH)])
                    cs_map[c] = (cosc, sinc)
                ems.append(cs_load)

                def kstep(k, W, wsrc, acc, out_of, passi):
                    def emit():
                        if c == 0 and wsrc is not None:
                            nc.sync.dma_start(out=W[k][:],
                                              in_=wsrc[ts(k, 128), :])
                        g = (c * 3 + passi) * KT_TILES + k
                        ht = ht_fetch(g)
                        st_, sp_ = (k == 0), (k == KT_TILES - 1)
                        if k == 0:
                            for d in range(HPC):
                                acc[d] = pp.tile([128, CH], F32,
                                                 tag="acc", bufs=4,
                                                 name=f"{out_of}{d}")
                        if out_of == "vps":
                            for jl in range(HPC):
                                mm(acc[jl][:], ht[:, ts(jl, 128)],
                                   W[k][:], st_, sp_)
                        else:
                            for d in range(HPC):
                                mm(acc[d][:], W[k][:, ts(d, 128)],
                                   ht[:], st_, sp_)
                    return emit

                # K pass + rope-K
                for k in range(KT_TILES):
                    ems.append(kstep(k, WK, wk, kps, "kps", 0))

                def ropek(d):
                    def emit():
                        rope(kps[d], KT[d][:, ts(c, CH)], c)
                    return emit
                for d in range(HPC):
                    ems.append(ropek(d))

                # Q pass + rope-Q
                for k in range(KT_TILES):
                    ems.append(kstep(k, WQ, wq, qps, "qps", 1))

                def ropeq(d):
                    def emit():
                        if d == 0:
                            QTc_map[c] = [
                                qtp.tile([128, CH], BF16, tag="qtc",
                                         bufs=2 * HPC, name=f"QTc{i}")
                                for i in range(HPC)]
                        rope(qps[d], QTc_map[c][d][:], c)
                    return emit
                for d in range(HPC):
                    ems.append(ropeq(d))

                # V pass + evict
                for k in range(KT_TILES):
                    ems.append(kstep(k, WV, wv, vps, "vps", 2))

                def vevict(jl):
                    def emit():
                        nc.scalar.copy(out=V[4 * c + jl][:],
                                       in_=vps[jl][:])
                    return emit
                for jl in range(HPC):
                    ems.append(vevict(jl))
                return ems

            # ---------------- attention emitters for one chunk ----------
            def make_head_emitters(c, h):
                jmax = 4 * c + 4 if causal else SJT
                st = {}

                def emit_pv(j):
                    mm(st['o'][:], V[j][:, ts(h, 128)], st['es'][j][:],
                       j == 0, j == jmax - 1)

                def block(j):
                    def emit():
                        if j == 0:
                            st['acc'] = accp.tile([128, CH], F32R,
                                                  tag="esacc", bufs=1,
                                                  name="esacc")
                            st['o'] = pp.tile([128, CH], F32, tag="ov",
                                              bufs=1, name="o_ps")
                            st['es'] = {}
                        s = pp.tile([128, CH], F32, tag="sc", bufs=2,
                                    name="s_ps")
                        es = esp.tile([128, CH], BF16, tag="es", bufs=2,
                                      name="es")
                        p = j - 4 * c if causal else -1
                        mm(s[:], KT[h][:, ts(j, 128)],
                           QTc_map[c][h][:], True, True)
                        if causal and p >= 0:
                            if p > 0:
                                nc.vector.memset(es[:, 0:p * 128], 0.0)
                            nc.vector.tensor_add(
                                s[:, ts(p, 128)], s[:, ts(p, 128)],
                                mtri_t[:, ts(p, 128)])
                            nc.scalar.activation(es[:, p * 128:],
                                                 s[:, p * 128:], EXPF)
                        else:
                            if not causal:
                                mg = mp.tile([128, CH], F32, tag="mg",
                                             bufs=3, name="mg")
                                nc.sync.dma_start(
                                    out=mg[:],
                                    in_=maskT[ts(j, 128), ts(c, CH)])
                                nc.vector.tensor_add(s[:], s[:], mg[:])
                            nc.scalar.activation(es[:], s[:], EXPF)
                        if j == 0:
                            nc.vector.tensor_copy(st['acc'][:], es[:])
                        elif j < jmax - 1:
                            # last block's es feeds the sums matmul directly
                            nc.vector.tensor_add(st['acc'][:],
                                                 st['acc'][:], es[:])
                        st['es'][j] = es
                        if j > 0:
                            emit_pv(j - 1)
                    return emit

                def tail1():
                    emit_pv(jmax - 1)
                    sums = pp.tile([128, CH], F32, tag="sc", bufs=2,
                                   name="sums")
                    mm(sums[0:1, :], ones_col[:], st['acc'][:],
                       True, False)
                    mm(sums[0:1, :], ones_col_bf[:],
                       st['es'][jmax - 1][:], False, True)
                    ssb = accp.tile([1, CH], F32R, tag="ssb", bufs=1,
                                    name="ssb")
                    nc.scalar.copy(out=ssb[:], in_=sums[0:1, :])
                    st['ssb'] = ssb

                def tail2():
                    # broadcast raw sums to 128 partitions, then take the
                    # reciprocal on DVE (off the PE dependency chain)
                    b_ps = pp.tile([128, CH], F32, tag="sc", bufs=2,
                                   name="b_ps")
                    mm(b_ps[:], ones_row[:], st['ssb'][:], True, True)
                    rb = accp.tile([128, CH], F32R, tag="rb", bufs=1,
                                   name="rb")
                    with nc.allow_low_precision(reason="softmax recip"):
                        nc.vector.reciprocal(rb[:], b_ps[:])
                    att = attp.tile([128, CH], BF16, tag="attT", bufs=8,
                                    name="att")
                    nc.vector.tensor_mul(att[:], st['o'][:], rb[:])
                    attT_map[(c, h)] = att

                return [block(j) for j in range(jmax)] + [tail1], tail2

            def make_attn_emitters(c):
                # weave each head's tail2 after the NEXT head's first block
                # (but before its second, which allocates/needs o_ps) so the
                # reciprocal chain latency is covered by PE work.
                ems = []
                carry = None
                for h in range(HPC):
                    head, t2 = make_head_emitters(c, h)
                    ems.append(head[0])
                    if carry is not None:
                        ems.append(carry)
                    ems += head[1:]
                    carry = t2
                ems.append(carry)
                return ems

            # ---------------- o_proj emitters for one chunk -------------
            def make_oproj_emitters(cc, tag):
                ats = [attT_map[(cc, h)] for h in range(HPC)]
                bufs = 4 if tag == "acc" else 1

                def otile(n):
                    def emit():
                        pps = pp.tile([128, CH], F32, tag=tag, bufs=bufs,
                                      name="pps")
                        for kl in range(HPC):
                            mm(pps[:], WO[kl][:, ts(n, 128)], ats[kl][:],
                               kl == 0, kl == HPC - 1)
                        ot = potp.tile([128, CH], BF16, tag="pot", bufs=2,
                                       name="ot")
                        if n % 2 == 0:
                            nc.scalar.copy(out=ot[:], in_=pps[:])
                        else:
                            nc.vector.tensor_copy(ot[:], pps[:])
                        nc.sync.dma_start(out=po[ts(n, 128), ts(cc, CH)],
                                          in_=ot[:])
                    return emit
                return [otile(n) for n in range(H // 128)]

            def interleave(spine, fillers):
                fi = 0
                for i, em in enumerate(spine):
                    em()
                    tgt = (i + 1) * len(fillers) // len(spine)
                    while fi < tgt:
                        fillers[fi]()
                        fi += 1
                while fi < len(fillers):
                    fillers[fi]()
                    fi += 1

            # ---------------- software-pipelined schedule ----------------
            oproj_fns = {}
            for c in range(NCH):
                fill = []
                if c >= 1:
                    fill = make_attn_emitters(c - 1)
                if c >= 2:
                    fill = _fair_merge(fill,
                                       make_oproj_emitters(c - 2, "pps"))
                interleave(make_pass_emitters(c), fill)

            # tail: attention(3) x o_proj(2), then o_proj(3)
            interleave(make_attn_emitters(NCH - 1),
                       make_oproj_emitters(NCH - 2, "acc"))
            for em in make_oproj_emitters(NCH - 1, "acc"):
                em()

    nc.compile()
    return nc


_CACHE = {}


def _get_nc(causal):
    if causal not in _CACHE:
        _CACHE[causal] = _build(causal)
    return _CACHE[causal]


def kernel(hidden_states, attention_mask, position_ids, Wq, Wk, Wv, Wo):
    global last_exec_time_ns
    B, S_, H_ = hidden_states.shape
    assert (B, S_, H_) == (1, S, H)
    hs = np.asarray(hidden_states, dtype=np.float32)
    mask = np.asarray(attention_mask, dtype=np.float32)[0, 0]
    pos = np.asarray(position_ids)[0].astype(np.float64)

    iu = np.triu_indices(S, k=1)
    il = np.tril_indices(S, k=0)
    causal = bool(np.all(mask[il] == 0.0) and np.all(mask[iu] <= -1e30))

    hT_b = _b(hs[0].T)
    scale = 1.0 / np.sqrt(D)

    inv_freq = 1.0 / (ROPE_BASE ** (np.arange(0, D, 2, dtype=np.float64) / D))
    ang = pos[None, :] * np.concatenate([inv_freq, inv_freq])[:, None]  # [D,S]
    cosT = _r(np.cos(ang))
    sgn = np.ones((D, 1)); sgn[: D // 2] = -1.0
    sinTs = _r(np.sin(ang) * sgn)

    if causal:
        # 4 diagonal-block triangle patterns packed into [128, 512]:
        # pattern p in cols [128p, 128p+128), NEG where key-row r > query-col
        mtri = np.zeros((128, CH), dtype=np.float32)
        rr = np.arange(128)[:, None]
        qq = np.arange(128)[None, :]
        for p in range(4):
            blk = mtri[:, p * 128:(p + 1) * 128]
            blk[rr > qq] = NEG
        mtri = _r(mtri)
    else:
        maskT = _r(mask.T)

    nc = _get_nc(causal)
    in_maps = []
    for c in range(N_CORES):
        sl = slice(c * HC, (c + 1) * HC)
        m = {
            "hT": hT_b,
            "wq": _b(np.asarray(Wq, np.float64)[:, sl] * scale),
            "wk": _b(np.asarray(Wk)[:, sl]),
            "wv": _b(np.asarray(Wv)[:, sl]),
            "wo": _b(np.asarray(Wo)[sl, :]),
            "cosT": cosT,
            "sinTs": sinTs,
        }
        if causal:
            m["mtri"] = mtri
        else:
            m["maskT"] = maskT
        in_maps.append(m)

    trace = bool(int(os.environ.get("BASS_KERNEL_TRACE", "0")))
    kw = {}
    if trace:
        kw["trace"] = True
        kw["tmpdir"] = os.environ.get("BASS_KERNEL_TRACE_DIR") or None
    res = run_bass_kernel_spmd(nc, in_maps, list(range(N_CORES)), **kw)
    last_exec_time_ns = res.exec_time_ns

    acc = np.zeros((H, S), dtype=np.float32)
    for c in range(N_CORES):
        acc += np.asarray(res.results[c]["po"], dtype=np.float32)
    out = acc.T.reshape(1, S, H)
    return out


# revision 26
# speedup vs baseline: 1.0618x; 1.0023x over previous
"""Trainium2 Bass kernel for LlamaAttention (B=1, S=2048, H=4096, 32 heads).

Tensor-parallel over heads: 8 cores x 4 heads. v4 design:
  - All matmuls bf16 (1 cycle/row, ldweights ~99ns fully hidden),
    accumulation in fp32 PSUM.
  - Wq/Wk/Wv/Wo slices resident in SBUF, loaded once during chunk 0.
  - Software pipeline across 512-wide seq chunks:
      chunk c projection passes (K -> rope-K, Q -> rope-Q, V) are emitted
      interleaved with attention(c-1) blocks and o_proj(c-2) tiles, so the
      tensor engine always has dense matmul work while exp (ACT), softmax
      sums (DVE) and rope (ACT+DMA+DVE) latencies play out on other engines.
      Tail: attention(3) x o_proj(2), then o_proj(3) on the freed acc ring.
  - RoPE rotate-half: ACT evict to SBUF + DMA partition shift + DVE combine.
  - Softmax: es accumulated on DVE, partition-reduced by a ones-matmul,
    1/sum broadcast by a rank-1 matmul (both inside the sc PSUM ring).
  - Causal staircase: diagonal blocks only mask/exp the valid columns.
  - po partials written bf16; host sums the 8 partials in fp32.
PSUM banks: acc=4 (kps/qps/vps + tail o_proj), sc=2, ov=1, pps=1.
"""

import os
import sys

if "/opt/trn_rl_repo" not in sys.path:
    sys.path.insert(0, "/opt/trn_rl_repo")

import numpy as np
import ml_dtypes

from concourse import bacc, mybir, tile
from concourse import bass
from concourse.bass_utils import run_bass_kernel_spmd

F32 = mybir.dt.float32
F32R = mybir.dt.float32r
BF16 = mybir.dt.bfloat16
EXPF = mybir.ActivationFunctionType.Exp

N_CORES = 8
S = 2048
H = 4096
N_HEADS = 32
D = 128
HPC = N_HEADS // N_CORES     # 4 heads per core
HC = HPC * D                 # 512
CH = 512
NCH = S // CH                # 4
KT_TILES = H // 128          # 32
SJT = S // 128               # 16
ROPE_BASE = 10000.0
NEG = -1.0e9

last_exec_time_ns = None

BFDT = ml_dtypes.bfloat16


def _r(x):
    return np.ascontiguousarray(x, dtype=np.float32)


def _b(x):
    return np.ascontiguousarray(np.asarray(x, dtype=np.float32), dtype=BFDT)


def _fair_merge(a, b):
    """Round-robin merge two emitter lists proportionally."""
    out = []
    na, nb = len(a), len(b)
    ia = ib = 0
    for _ in range(na + nb):
        if ib * na <= ia * nb and ib < nb:
            out.append(b[ib]); ib += 1
        elif ia < na:
            out.append(a[ia]); ia += 1
        else:
            out.append(b[ib]); ib += 1
    return out


def _build(causal: bool):
    nc = bacc.Bacc("TRN2", target_bir_lowering=False, debug=False,
                   num_devices=N_CORES)
    hT = nc.dram_tensor("hT", [H, S], BF16, kind="ExternalInput")
    wq = nc.dram_tensor("wq", [H, HC], BF16, kind="ExternalInput")
    wk = nc.dram_tensor("wk", [H, HC], BF16, kind="ExternalInput")
    wv = nc.dram_tensor("wv", [H, HC], BF16, kind="ExternalInput")
    wo = nc.dram_tensor("wo", [HC, H], BF16, kind="ExternalInput")
    cosT = nc.dram_tensor("cosT", [D, S], F32, kind="ExternalInput")
    sinTs = nc.dram_tensor("sinTs", [D, S], F32, kind="ExternalInput")
    if causal:
        mtri = nc.dram_tensor("mtri", [128, CH], F32, kind="ExternalInput")
    else:
        maskT = nc.dram_tensor("maskT", [S, S], F32, kind="ExternalInput")
    po = nc.dram_tensor("po", [H, S], BF16, kind="ExternalOutput")

    def mm(out, lhsT, rhs, start, stop, skip=False):
        nc.tensor.matmul(out, lhsT, rhs, start=start, stop=stop,
                         skip_group_check=skip)

    ts = bass.ts

    with tile.TileContext(nc) as tc:
        with tc.tile_pool(name="wkp", bufs=KT_TILES) as wkp, \
             tc.tile_pool(name="wqp", bufs=KT_TILES) as wqp, \
             tc.tile_pool(name="wvp", bufs=KT_TILES) as wvp, \
             tc.tile_pool(name="wop", bufs=HPC) as wop, \
             tc.tile_pool(name="ktp", bufs=HPC) as ktp, \
             tc.tile_pool(name="vpool", bufs=SJT) as vpool, \
             tc.tile_pool(name="csp", bufs=2) as csp, \
             tc.tile_pool(name="htp", bufs=7) as htp, \
             tc.tile_pool(name="qtp", bufs=2 * HPC) as qtp, \
             tc.tile_pool(name="ropep", bufs=2) as ropep, \
             tc.tile_pool(name="esp", bufs=2) as esp, \
             tc.tile_pool(name="accp", bufs=4) as accp, \
             tc.tile_pool(name="attp", bufs=8) as attp, \
             tc.tile_pool(name="potp", bufs=2) as potp, \
             tc.tile_pool(name="mp", bufs=3) as mp, \
             tc.tile_pool(name="pp", bufs=8, space="PSUM") as pp:

            WK = [wkp.tile([128, HC], BF16, tag="wk", name=f"WK{k}")
                  for k in range(KT_TILES)]
            WQ = [wqp.tile([128, HC], BF16, tag="wq", name=f"WQ{k}")
                  for k in range(KT_TILES)]
            WV = [wvp.tile([128, HC], BF16, tag="wv", name=f"WV{k}")
                  for k in range(KT_TILES)]
            WO = [wop.tile([128, H], BF16, tag="wo", name=f"WO{k}")
                  for k in range(HPC)]
            KT = [ktp.tile([128, S], BF16, tag="kt", name=f"KT{h}")
                  for h in range(HPC)]
            V = [vpool.tile([128, HC], BF16, tag="v", name=f"V{j}")
                 for j in range(SJT)]
            if causal:
                mtri_t = mp.tile([128, CH], F32, tag="mtri", bufs=1,
                                 name="mtri_t")
                nc.sync.dma_start(out=mtri_t[:], in_=mtri[:, :])

            ones_col32 = mp.tile([128, 1], F32, tag="oc32", bufs=1,
                                 name="ones_col32")
            ones_row32 = mp.tile([1, 128], F32, tag="or32", bufs=1,
                                 name="ones_row32")
            nc.vector.memset(ones_col32[:], 1.0)
            nc.vector.memset(ones_row32[:], 1.0)
            ones_col = mp.tile([128, 1], F32R, tag="oc", bufs=1,
                              name="ones_col")
            ones_row = mp.tile([1, 128], F32R, tag="or", bufs=1,
                              name="ones_row")
            ones_col_bf = mp.tile([128, 1], BF16, tag="ocb", bufs=1,
                                  name="ones_col_bf")
            nc.vector.tensor_copy(ones_col[:], ones_col32[:])
            nc.vector.tensor_copy(ones_row[:], ones_row32[:])
            nc.vector.tensor_copy(ones_col_bf[:], ones_col32[:])

            for kl in range(HPC):
                nc.sync.dma_start(out=WO[kl][:], in_=wo[ts(kl, 128), :])

            attT_map = {}
            QTc_map = {}
            cs_map = {}
            # rolling ht prefetch stream across all passes and chunks:
            # global step g in [0, NCH*96): chunk g//96, k-tile g%32
            ht_stream = {}
            HT_LEAD = 5
            HT_STEPS = NCH * 3 * KT_TILES

            def ht_fetch(g):
                for gg in range(g, min(g + HT_LEAD + 1, HT_STEPS)):
                    if gg not in ht_stream:
                        cc = gg // (3 * KT_TILES)
                        kk = gg % KT_TILES
                        t = htp.tile([128, CH], BF16, tag="ht", bufs=7,
                                     name="ht")
                        nc.sync.dma_start(
                            out=t[:], in_=hT[ts(kk, 128), ts(cc, CH)])
                        ht_stream[gg] = t
                return ht_stream.pop(g)

            def rope(ps, dst_ap, c):
                cosc, sinc = cs_map[c]
                raw = ropep.tile([128, CH], F32, tag="raw", bufs=1,
                                 name="raw")
                nc.scalar.copy(out=raw[:], in_=ps[:])
                shf = ropep.tile([128, CH], F32, tag="shf", bufs=1,
                                 name="shf")
                nc.sync.dma_start(out=shf[0:64, :], in_=raw[64:128, :])
                nc.sync.dma_start(out=shf[64:128, :], in_=raw[0:64, :])
                tmp = ropep.tile([128, CH], F32, tag="rtmp", bufs=1,
                                 name="rtmp")
                nc.vector.tensor_mul(tmp[:], shf[:], sinc[:])
                nc.vector.tensor_mul(dst_ap, raw[:], cosc[:])
                nc.vector.tensor_add(dst_ap, dst_ap, tmp[:])

            # ---------------- pass emitters for one chunk ----------------
            def make_pass_emitters(c):
                ems = []
                kps = [None] * HPC
                qps = [None] * HPC
                vps = [None] * HPC

                def cs_load():
                    cosc = csp.tile([128, CH], F32, tag="cos", bufs=1,
                                    name="cosc")
                    sinc = csp.tile([128, CH], F32, tag="sin", bufs=1,
                                    name="sinc")
                    nc.sync.dma_start(out=cosc[:], in_=cosT[:, ts(c, CH)])
                    nc.sync.dma_start(out=sinc[:], in_=sinTs[:, ts(c, CH)])
                    cs_map[c] = (cosc, sinc)
                ems.append(cs_load)

                def kstep(k, W, wsrc, acc, out_of, passi):
                    def emit():
                        if c == 0 and wsrc is not None:
                            nc.sync.dma_start(out=W[k][:],
                                              in_=wsrc[ts(k, 128), :])
                        g = (c * 3 + passi) * KT_TILES + k
                        ht = ht_fetch(g)
                        st_, sp_ = (k == 0), (k == KT_TILES - 1)
                        if k == 0:
                            for d in range(HPC):
                                acc[d] = pp.tile([128, CH], F32,
                                                 tag="acc", bufs=4,
                                                 name=f"{out_of}{d}")
                        if out_of == "vps":
                            for jl in range(HPC):
                                mm(acc[jl][:], ht[:, ts(jl, 128)],
                                   W[k][:], st_, sp_)
                        else:
                            for d in range(HPC):
                                mm(acc[d][:], W[k][:, ts(d, 128)],
                                   ht[:], st_, sp_)
                    return emit

                # K pass + rope-K
                for k in range(KT_TILES):
                    ems.append(kstep(k, WK, wk, kps, "kps", 0))

                def ropek(d):
                    def emit():
                        rope(kps[d], KT[d][:, ts(c, CH)], c)
                    return emit
                for d in range(HPC):
                    ems.append(ropek(d))

                # Q pass + rope-Q
                for k in range(KT_TILES):
                    ems.append(kstep(k, WQ, wq, qps, "qps", 1))

                def ropeq(d):
                    def emit():
                        if d == 0:
                            QTc_map[c] = [
                                qtp.tile([128, CH], BF16, tag="qtc",
                                         bufs=2 * HPC, name=f"QTc{i}")
                                for i in range(HPC)]
                        rope(qps[d], QTc_map[c][d][:], c)
                    return emit
                for d in range(HPC):
                    ems.append(ropeq(d))

                # V pass + evict
                for k in range(KT_TILES):
                    ems.append(kstep(k, WV, wv, vps, "vps", 2))

                def vevict(jl):
                    def emit():
                        nc.scalar.copy(out=V[4 * c + jl][:],
                                       in_=vps[jl][:])
                    return emit
                for jl in range(HPC):
                    ems.append(vevict(jl))
                return ems

            # ---------------- attention emitters for one chunk ----------
            def make_head_emitters(c, h):
                jmax = 4 * c + 4 if causal else SJT
                st = {}

                def emit_pv(j):
                    mm(st['o'][:], V[j][:, ts(h, 128)], st['es'][j][:],
                       j == 0, j == jmax - 1)

                def block(j):
                    def emit():
                        if j == 0:
                            st['acc'] = accp.tile([128, CH], F32R,
                                                  tag="esacc", bufs=1,
                                                  name="esacc")
                            st['o'] = pp.tile([128, CH], F32, tag="ov",
                                              bufs=1, name="o_ps")
                            st['es'] = {}
                        s = pp.tile([128, CH], F32, tag="sc", bufs=2,
                                    name="s_ps")
                        es = esp.tile([128, CH], BF16, tag="es", bufs=2,
                                      name="es")
                        p = j - 4 * c if causal else -1
                        mm(s[:], KT[h][:, ts(j, 128)],
                           QTc_map[c][h][:], True, True)
                        if causal and p >= 0:
                            if p > 0:
                                nc.vector.memset(es[:, 0:p * 128], 0.0)
                            nc.vector.tensor_add(
                                s[:, ts(p, 128)], s[:, ts(p, 128)],
                                mtri_t[:, ts(p, 128)])
                            nc.scalar.activation(es[:, p * 128:],
                                                 s[:, p * 128:], EXPF)
                        else:
                            if not causal:
                                mg = mp.tile([128, CH], F32, tag="mg",
                                             bufs=3, name="mg")
                                nc.sync.dma_start(
                                    out=mg[:],
                                    in_=maskT[ts(j, 128), ts(c, CH)])
                                nc.vector.tensor_add(s[:], s[:], mg[:])
                            nc.scalar.activation(es[:], s[:], EXPF)
                        if j == 0:
                            nc.vector.tensor_copy(st['acc'][:], es[:])
                        elif j < jmax - 1:
                            # last block's es feeds the sums matmul directly
                            nc.vector.tensor_add(st['acc'][:],
                                                 st['acc'][:], es[:])
                        st['es'][j] = es
                        if j > 0:
                            emit_pv(j - 1)
                    return emit

                def tail1():
                    emit_pv(jmax - 1)
                    sums = pp.tile([128, CH], F32, tag="sc", bufs=2,
                                   name="sums")
                    mm(sums[0:1, :], ones_col[:], st['acc'][:],
                       True, False)
                    mm(sums[0:1, :], ones_col_bf[:],
                       st['es'][jmax - 1][:], False, True)
                    ssb = accp.tile([1, CH], F32R, tag="ssb", bufs=1,
                                    name="ssb")
                    nc.scalar.copy(out=ssb[:], in_=sums[0:1, :])
                    st['ssb'] = ssb

                def tail2():
                    # broadcast raw sums to 128 partitions, then take the
                    # reciprocal on DVE (off the PE dependency chain)
                    b_ps = pp.tile([128, CH], F32, tag="sc", bufs=2,
                                   name="b_ps")
                    mm(b_ps[:], ones_row[:], st['ssb'][:], True, True)
                    rb = accp.tile([128, CH], F32R, tag="rb", bufs=1,
                                   name="rb")
                    with nc.allow_low_precision(reason="softmax recip"):
                        nc.vector.reciprocal(rb[:], b_ps[:])
                    att = attp.tile([128, CH], BF16, tag="attT", bufs=8,
                                    name="att")
                    nc.vector.tensor_mul(att[:], st['o'][:], rb[:])
                    attT_map[(c, h)] = att

                return [block(j) for j in range(jmax)] + [tail1], tail2

            def make_attn_emitters(c):
                # weave each head's tail2 after the NEXT head's first block
                # (but before its second, which allocates/needs o_ps) so the
                # reciprocal chain latency is covered by PE work.
                ems = []
                carry = None
                for h in range(HPC):
                    head, t2 = make_head_emitters(c, h)
                    ems.append(head[0])
                    if carry is not None:
                        ems.append(carry)
                    ems += head[1:]
                    carry = t2
                ems.append(carry)
                return ems

            # ---------------- o_proj emitters for one chunk -------------
            def make_oproj_emitters(cc, tag):
                ats = [attT_map[(cc, h)] for h in range(HPC)]
                bufs = 4 if tag == "acc" else 1

                def otile(n):
                    def emit():
                        pps = pp.tile([128, CH], F32, tag=tag, bufs=bufs,
                                      name="pps")
                        for kl in range(HPC):
                            mm(pps[:], WO[kl][:, ts(n, 128)], ats[kl][:],
                               kl == 0, kl == HPC - 1)
                        ot = potp.tile([128, CH], BF16, tag="pot", bufs=2,
                                       name="ot")
                        if n % 2 == 0:
                            nc.scalar.copy(out=ot[:], in_=pps[:])
                        else:
                            nc.vector.tensor_copy(ot[:], pps[:])
                        nc.sync.dma_start(out=po[ts(n, 128), ts(cc, CH)],
                                          in_=ot[:])
                    return emit
                return [otile(n) for n in range(H // 128)]

            def interleave(spine, fillers, nofill=()):
                fi = 0
                for i, em in enumerate(spine):
                    em()
                    if i in nofill:
                        continue
                    tgt = (i + 1) * len(fillers) // len(spine)
                    while fi < tgt:
                        fillers[fi]()
                        fi += 1
                while fi < len(fillers):
                    fillers[fi]()
                    fi += 1

            # ---------------- software-pipelined schedule ----------------
            oproj_fns = {}
            for c in range(NCH):
                fill = []
                if c >= 1:
                    fill = make_attn_emitters(c - 1)
                if c >= 2:
                    fill = _fair_merge(fill,
                                       make_oproj_emitters(c - 2, "pps"))
                # hold fillers back around pass ends so the ACT queue
                # is drained when rope evictions need to free PSUM banks
                nofill = set()
                for base in (1, 37, 73):        # k-loop start offsets
                    nofill.update(range(base + 29, base + 36))
                interleave(make_pass_emitters(c), fill, nofill)

            # tail: attention(3) x o_proj(2), then o_proj(3)
            interleave(make_attn_emitters(NCH - 1),
                       make_oproj_emitters(NCH - 2, "acc"))
            for em in make_oproj_emitters(NCH - 1, "acc"):
                em()

    nc.compile()
    return nc


_CACHE = {}


def _get_nc(causal):
    if causal not in _CACHE:
        _CACHE[causal] = _build(causal)
    return _CACHE[causal]


def kernel(hidden_states, attention_mask, position_ids, Wq, Wk, Wv, Wo):
    global last_exec_time_ns
    B, S_, H_ = hidden_states.shape
    assert (B, S_, H_) == (1, S, H)
    hs = np.asarray(hidden_states, dtype=np.float32)
    mask = np.asarray(attention_mask, dtype=np.float32)[0, 0]
    pos = np.asarray(position_ids)[0].astype(np.float64)

    iu = np.triu_indices(S, k=1)
    il = np.tril_indices(S, k=0)
    causal = bool(np.all(mask[il] == 0.0) and np.all(mask[iu] <= -1e30))

    hT_b = _b(hs[0].T)
    scale = 1.0 / np.sqrt(D)

    inv_freq = 1.0 / (ROPE_BASE ** (np.arange(0, D, 2, dtype=np.float64) / D))
    ang = pos[None, :] * np.concatenate([inv_freq, inv_freq])[:, None]  # [D,S]
    cosT = _r(np.cos(ang))
    sgn = np.ones((D, 1)); sgn[: D // 2] = -1.0
    sinTs = _r(np.sin(ang) * sgn)

    if causal:
        # 4 diagonal-block triangle patterns packed into [128, 512]:
        # pattern p in cols [128p, 128p+128), NEG where key-row r > query-col
        mtri = np.zeros((128, CH), dtype=np.float32)
        rr = np.arange(128)[:, None]
        qq = np.arange(128)[None, :]
        for p in range(4):
            blk = mtri[:, p * 128:(p + 1) * 128]
            blk[rr > qq] = NEG
        mtri = _r(mtri)
    else:
        maskT = _r(mask.T)

    nc = _get_nc(causal)
    in_maps = []
    for c in range(N_CORES):
        sl = slice(c * HC, (c + 1) * HC)
        m = {
            "hT": hT_b,
            "wq": _b(np.asarray(Wq, np.float64)[:, sl] * scale),
            "wk": _b(np.asarray(Wk)[:, sl]),
            "wv": _b(np.asarray(Wv)[:, sl]),
            "wo": _b(np.asarray(Wo)[sl, :]),
            "cosT": cosT,
            "sinTs": sinTs,
        }
        if causal:
            m["mtri"] = mtri
        else:
            m["maskT"] = maskT
        in_maps.append(m)

    trace = bool(int(os.environ.get("BASS_KERNEL_TRACE", "0")))
    kw = {}
    if trace:
        kw["trace"] = True
        kw["tmpdir"] = os.environ.get("BASS_KERNEL_TRACE_DIR") or None
    res = run_bass_kernel_spmd(nc, in_maps, list(range(N_CORES)), **kw)
    last_exec_time_ns = res.exec_time_ns

    acc = np.zeros((H, S), dtype=np.float32)
    for c in range(N_CORES):
        acc += np.asarray(res.results[c]["po"], dtype=np.float32)
    out = acc.T.reshape(1, S, H)
    return out
